# revision 1
# baseline (speedup 1.0000x reference)
"""Trainium2 Bass kernel for nn_BDHAttention (RoPE(Q) self-score attention, no softmax).

Per (batch, head) slice s: QR = rope(Q_s) [T,N]; S = QR @ QR.T / sqrt(N) [T,T];
O_s = S @ V_s [T,N].  K input is unused by the reference.  B*nh = 8 slices map
1:1 onto the 8 NeuronCores (data/head parallel, no communication).

Device-side structure per core (T=2048, N=4096, P=128):
  - RoPE on DVE in [t-partition, n-free] layout using host-precomputed fp16
    cos/sin tables (scaled by 1/8 each so S picks up the 1/64 scale for free);
    output QR' in fp16.
  - PE-transpose QR' 128x128 tiles to build QR'^T panels (fp16).
  - MM1 (fp16, fp32 PSUM accum): S[A,A] from panel A; S[B,B], S[A,B] from
    panel B + panel A streamed back from a DRAM scratch copy; S[B,A] filled by
    PE-transposing S[A,B] blocks (S is symmetric).  S stored fp16 in DRAM.
  - MM2: O = S @ V.  S row-panels re-read from DRAM serve directly as lhsT
    tiles (partition = contraction dim) thanks to S's symmetry; V streamed
    fp16; O accumulated fp32 in PSUM and written out fp32.
"""

import math
import sys

sys.path.insert(0, "/opt/trn_rl_repo")

import numpy as np

import concourse.bacc as bacc
import concourse.mybir as mybir
import concourse.tile as tile
from concourse.bass_utils import run_bass_kernel_spmd

B, NH, T, N = 2, 4, 2048, 4096
THETA = 2 ** 16
P = 128
HALF = T // 2            # 1024
NTILES = T // P          # 16 t-tiles
NCH = N // P             # 32 n-chunks
F = 512                  # matmul moving free dim (one fp32 PSUM bank)

f16 = mybir.dt.float16
f32 = mybir.dt.float32


def _build_nc():
    nc = bacc.Bacc("TRN2", target_bir_lowering=False, debug=False, num_devices=8)

    q = nc.dram_tensor("q", [T, N], f32, kind="ExternalInput")
    v = nc.dram_tensor("v", [T, N], f16, kind="ExternalInput")
    cu = nc.dram_tensor("cu", [T, N // 2], f16, kind="ExternalInput")
    su = nc.dram_tensor("su", [T, N // 2], f16, kind="ExternalInput")
    ident = nc.dram_tensor("ident", [P, P], f16, kind="ExternalInput")
    o = nc.dram_tensor("o", [T, N], f32, kind="ExternalOutput")

    with tile.TileContext(nc) as tc:
        with (
            tc.tile_pool(name="dram", bufs=1, space="DRAM") as dram,
            tc.tile_pool(name="const", bufs=1) as const,
            tc.tile_pool(name="panel", bufs=1) as panel,
            tc.tile_pool(name="ps", bufs=1, space="PSUM") as ps,
            tc.tile_pool(name="work", bufs=1) as work,
        ):
            qrt_a = dram.tile([N, HALF], f16, name="qrt_a")
            s_mat = dram.tile([T, T], f16, name="s_mat")

            idt = const.tile([P, P], f16, name="idt")
            nc.sync.dma_start(idt[:], ident.ap())

            def build_panel(half):
                """RoPE + transpose t-tiles [half*HALF, (half+1)*HALF) into
                panel tiles pk_<k> (QR'^T[n-chunk k, t within half])."""
                tiles = []
                for k in range(NCH):
                    t_ = panel.tile([P, HALF], f16, name=f"pk_{k}", tag=f"pk_{k}")
                    tiles.append(t_)
                for ti in range(NTILES // 2):
                    trow = half * (NTILES // 2) + ti
                    qt = work.tile([P, N], f32, name="qt", tag="qt", bufs=2)
                    cut = work.tile([P, N // 2], f16, name="cut", tag="cut", bufs=2)
                    sut = work.tile([P, N // 2], f16, name="sut", tag="sut", bufs=2)
                    nc.sync.dma_start(qt[:], q.ap()[trow * P:(trow + 1) * P, :])
                    nc.sync.dma_start(cut[:], cu.ap()[trow * P:(trow + 1) * P, :])
                    nc.sync.dma_start(sut[:], su.ap()[trow * P:(trow + 1) * P, :])
                    qr = work.tile([P, N], f16, name="qr", tag="qr", bufs=2)
                    t1 = work.tile([P, N // 2], f32, name="t1", tag="t1", bufs=2)
                    t2 = work.tile([P, N // 2], f32, name="t2", tag="t2", bufs=2)
                    q3 = qt[:].rearrange("p (i w) -> p i w", w=2)
                    qr3 = qr[:].rearrange("p (i w) -> p i w", w=2)
                    qe, qo = q3[:, :, 0], q3[:, :, 1]
                    nc.vector.tensor_mul(t1[:], qe, cut[:])
                    nc.vector.tensor_mul(t2[:], qo, sut[:])
                    nc.vector.tensor_sub(qr3[:, :, 0], t1[:], t2[:])
                    nc.vector.tensor_mul(t1[:], qo, cut[:])
                    nc.vector.tensor_mul(t2[:], qe, sut[:])
                    nc.vector.tensor_add(qr3[:, :, 1], t1[:], t2[:])
                    for k in range(NCH):
                        pt = ps.tile([P, P], f16, name="tr", tag="tr", bufs=2)
                        nc.tensor.transpose(pt[:], qr[:, k * P:(k + 1) * P], idt[:])
                        nc.scalar.copy(tiles[k][:, ti * P:(ti + 1) * P], pt[:])
                return tiles

            def s_block(psrc, m, fcol):
                """Evacuate one accumulated S block [P, F] (rows m*P, cols
                fcol*F of s_mat) and return the fp16 SBUF staging tile."""
                st = work.tile([P, F], f16, name="sst", tag="sst", bufs=3)
                nc.vector.tensor_copy(st[:], psrc[:])
                nc.sync.dma_start(
                    s_mat[m * P:(m + 1) * P, fcol * F:(fcol + 1) * F], st[:]
                )
                return st

            # ---- pass 0: panel A, S[A,A] ----
            pa = build_panel(0)
            for k in range(NCH):
                nc.sync.dma_start(qrt_a[k * P:(k + 1) * P, :], pa[k][:])
            for m in range(HALF // P):
                for fc in range(HALF // F):
                    acc = ps.tile([P, F], f32, name="acc", tag="acc", bufs=4)
                    for k in range(NCH):
                        nc.tensor.matmul(
                            acc[:],
                            pa[k][:, m * P:(m + 1) * P],
                            pa[k][:, fc * F:(fc + 1) * F],
                            start=(k == 0),
                            stop=(k == NCH - 1),
                        )
                    s_block(acc, m, fc)

            # ---- pass 1: panel B, S[B,B], S[A,B], fill S[B,A] ----
            pb = build_panel(1)
            for m in range(HALF // P):
                for fc in range(HALF // F):
                    acc = ps.tile([P, F], f32, name="acc", tag="acc", bufs=4)
                    for k in range(NCH):
                        nc.tensor.matmul(
                            acc[:],
                            pb[k][:, m * P:(m + 1) * P],
                            pb[k][:, fc * F:(fc + 1) * F],
                            start=(k == 0),
                            stop=(k == NCH - 1),
                        )
                    s_block(acc, HALF // P + m, HALF // F + fc)
            for m in range(HALF // P):
                accs = [
                    ps.tile([P, F], f32, name="acc", tag="acc", bufs=4)
                    for _ in range(HALF // F)
                ]
                for k in range(NCH):
                    la = work.tile([P, P], f16, name="la", tag="la", bufs=3)
                    nc.sync.dma_start(
                        la[:], qrt_a[k * P:(k + 1) * P, m * P:(m + 1) * P]
                    )
                    for fc in range(HALF // F):
                        nc.tensor.matmul(
                            accs[fc][:],
                            la[:],
                            pb[k][:, fc * F:(fc + 1) * F],
                            start=(k == 0),
                            stop=(k == NCH - 1),
                        )
                for fc in range(HALF // F):
                    st = s_block(accs[fc], m, HALF // F + fc)
                    # symmetric fill: S[B,A] sub-blocks = transpose of this block
                    for sub in range(F // P):
                        pt = ps.tile([P, P], f16, name="tr", tag="tr", bufs=2)
                        nc.tensor.transpose(
                            pt[:], st[:, sub * P:(sub + 1) * P], idt[:]
                        )
                        ft = work.tile([P, P], f16, name="ft", tag="ft", bufs=3)
                        nc.scalar.copy(ft[:], pt[:])
                        r0 = HALF + fc * F + sub * P
                        nc.sync.dma_start(
                            s_mat[r0:r0 + P, m * P:(m + 1) * P], ft[:]
                        )

            # ---- MM2: O = S @ V (S row-panels as lhsT via symmetry) ----
            srow = []
            for k in range(NTILES):
                u = panel.tile([P, HALF], f16, name=f"pk_{2 * k}", tag=f"pk_{2 * k}")
                w = panel.tile(
                    [P, HALF], f16, name=f"pk_{2 * k + 1}", tag=f"pk_{2 * k + 1}"
                )
                nc.sync.dma_start(u[:], s_mat[k * P:(k + 1) * P, 0:HALF])
                nc.sync.dma_start(w[:], s_mat[k * P:(k + 1) * P, HALF:T])
                srow.append((u, w))

            for j in range(N // F):
                vts = []
                for k in range(NTILES):
                    vt = work.tile([P, F], f16, name=f"vt_{k}", tag=f"vt_{k}", bufs=2)
                    nc.sync.dma_start(
                        vt[:], v.ap()[k * P:(k + 1) * P, j * F:(j + 1) * F]
                    )
                    vts.append(vt)
                for m in range(NTILES):
                    acc = ps.tile([P, F], f32, name="acc", tag="acc", bufs=4)
                    for k in range(NTILES):
                        u, w = srow[k]
                        lhsT = (
                            u[:, m * P:(m + 1) * P]
                            if m < 8
                            else w[:, (m - 8) * P:(m - 7) * P]
                        )
                        nc.tensor.matmul(
                            acc[:], lhsT, vts[k][:],
                            start=(k == 0), stop=(k == NTILES - 1),
                        )
                    ot = work.tile([P, F], f32, name="ot", tag="ot", bufs=3)
                    nc.scalar.copy(ot[:], acc[:])
                    nc.sync.dma_start(
                        o.ap()[m * P:(m + 1) * P, j * F:(j + 1) * F], ot[:]
                    )

    nc.compile()
    return nc


def _tables():
    idx = np.arange(N, dtype=np.float32)
    qq = np.floor(idx / 2.0) * 2.0
    freqs = (1.0 / THETA ** (qq / N) / (2.0 * math.pi)).astype(np.float32)
    fe = freqs[::2]  # [N/2], pairs share a frequency
    ph = (np.arange(T, dtype=np.float32)[:, None] * fe[None, :]).astype(np.float32)
    ang = (np.mod(ph, 1.0) * np.float32(2.0 * math.pi)).astype(np.float32)
    cu = (np.cos(ang.astype(np.float64)) / 8.0).astype(np.float16)
    su = (np.sin(ang.astype(np.float64)) / 8.0).astype(np.float16)
    return cu, su


_NC_CACHE = {}


def kernel(Q, K, V, _trace=False, _tmpdir=None):
    del K  # unused by the reference computation
    if "nc" not in _NC_CACHE:
        _NC_CACHE["nc"] = _build_nc()
    nc = _NC_CACHE["nc"]

    cu, su = _tables()
    ident = np.eye(P, dtype=np.float16)
    V16 = np.asarray(V, dtype=np.float16)
    Q32 = np.asarray(Q, dtype=np.float32)

    in_maps = []
    for c in range(8):
        b, h = divmod(c, NH)
        in_maps.append({
            "q": np.ascontiguousarray(Q32[b, h]),
            "v": np.ascontiguousarray(V16[b, h]),
            "cu": cu,
            "su": su,
            "ident": ident,
        })

    kw = {}
    if _trace:
        kw = dict(trace=True, tmpdir=_tmpdir)
    res = run_bass_kernel_spmd(nc, in_maps, list(range(8)), **kw)

    out = np.empty((B, NH, T, N), dtype=np.float32)
    for c in range(8):
        b, h = divmod(c, NH)
        out[b, h] = res.results[c]["o"]
    if _trace:
        kernel.last_exec_time_ns = res.exec_time_ns
    return out


# revision 3
# speedup vs baseline: 1.1999x; 1.1999x over previous
"""Trainium2 Bass kernel for nn_BDHAttention (RoPE(Q) self-score attention, no softmax).

Per (batch, head) slice s: QR = rope(Q_s) [T,N]; S = QR @ QR.T / sqrt(N) [T,T];
O_s = S @ V_s [T,N].  K input is unused by the reference.  B*nh = 8 slices map
1:1 onto the 8 NeuronCores (data/head parallel, no communication).

Device-side structure per core (T=2048, N=4096, P=128):
  - Q arrives fp16 with its feature dim de-interleaved on the host
    ([evens | odds]) so RoPE is all contiguous 16-bit tensor_tensor ops
    (DVE 2x mode).  The n-permutation is harmless: it is the contraction
    dim of S = QR @ QR.T and both operands share it.
  - cos/sin tables are host-precomputed fp16, scaled by 1/8 each so S picks
    up the 1/64 = 1/sqrt(N) scale for free.
  - PE-transpose QR' 128x128 tiles into two resident fp16 panels
    (QR'^T, t-halves A and B).  Panel-B build is interleaved with the
    S[A,A] matmuls to keep the PE dense (HAM stays warm).
  - MM1 (fp16, fp32 PSUM accum): S[A,A], S[B,B], S[A,B] all from resident
    panels; S[B,A] filled by PE-transposing S[A,B] blocks (S symmetric).
    S stored fp16 in a DRAM scratch.
  - MM2: O = S @ V.  S row-panels re-read from DRAM serve directly as lhsT
    tiles (partition = contraction dim) thanks to S's symmetry; V streamed
    fp16; O accumulated fp32 in PSUM and written out fp32.
"""

import math
import sys

sys.path.insert(0, "/opt/trn_rl_repo")

import numpy as np

import concourse.bacc as bacc
import concourse.mybir as mybir
import concourse.tile as tile
from concourse.bass_utils import run_bass_kernel_spmd

B, NH, T, N = 2, 4, 2048, 4096
THETA = 2 ** 16
P = 128
HALF = T // 2            # 1024
NTILES = T // P          # 16 t-tiles
NCH = N // P             # 32 n-chunks
F = 512                  # matmul moving free dim (one fp32 PSUM bank)
H = N // 2               # 2048

f16 = mybir.dt.float16
f32 = mybir.dt.float32


def _build_nc():
    nc = bacc.Bacc("TRN2", target_bir_lowering=False, debug=False, num_devices=8)

    q = nc.dram_tensor("q", [T, N], f16, kind="ExternalInput")
    v = nc.dram_tensor("v", [T, N], f16, kind="ExternalInput")
    cu = nc.dram_tensor("cu", [T, H], f16, kind="ExternalInput")
    su = nc.dram_tensor("su", [T, H], f16, kind="ExternalInput")
    ident = nc.dram_tensor("ident", [P, P], f16, kind="ExternalInput")
    o = nc.dram_tensor("o", [T, N], f32, kind="ExternalOutput")

    with tile.TileContext(nc) as tc:
        with (
            tc.tile_pool(name="dram", bufs=1, space="DRAM") as dram,
            tc.tile_pool(name="const", bufs=1) as const,
            tc.tile_pool(name="panel", bufs=1) as panel,
            tc.tile_pool(name="ps", bufs=1, space="PSUM") as ps,
            tc.tile_pool(name="work", bufs=1) as work,
        ):
            s_mat = dram.tile([T, T], f16, name="s_mat")

            idt = const.tile([P, P], f16, name="idt")
            nc.sync.dma_start(idt[:], ident.ap())

            pa = [
                panel.tile([P, HALF], f16, name=f"pk_a{k}", tag=f"pk_a{k}")
                for k in range(NCH)
            ]
            pb = [
                panel.tile([P, HALF], f16, name=f"pk_b{k}", tag=f"pk_b{k}")
                for k in range(NCH)
            ]

            def build_tile(dst, half, ti):
                """RoPE t-tile (half*8 + ti) and transpose its 32 n-chunks into
                panel columns ti*P:(ti+1)*P."""
                trow = half * (NTILES // 2) + ti
                qt = work.tile([P, N], f16, name="qt", tag="qt", bufs=1)
                cut = work.tile([P, H], f16, name="cut", tag="cut", bufs=1)
                sut = work.tile([P, H], f16, name="sut", tag="sut", bufs=1)
                nc.sync.dma_start(qt[:], q.ap()[trow * P:(trow + 1) * P, :])
                nc.sync.dma_start(cut[:], cu.ap()[trow * P:(trow + 1) * P, :])
                nc.sync.dma_start(sut[:], su.ap()[trow * P:(trow + 1) * P, :])
                qr = work.tile([P, N], f16, name="qr", tag="qr", bufs=1)
                t1 = work.tile([P, H], f16, name="t1", tag="t1", bufs=1)
                t2 = work.tile([P, H], f16, name="t2", tag="t2", bufs=1)
                qe, qo = qt[:, 0:H], qt[:, H:N]
                nc.vector.tensor_mul(t1[:], qe, cut[:])
                nc.vector.tensor_mul(t2[:], qo, sut[:])
                nc.vector.tensor_sub(qr[:, 0:H], t1[:], t2[:])
                nc.vector.tensor_mul(t1[:], qo, cut[:])
                nc.vector.tensor_mul(t2[:], qe, sut[:])
                nc.vector.tensor_add(qr[:, H:N], t1[:], t2[:])
                for k in range(NCH):
                    pt = ps.tile([P, P], f16, name="tr", tag="tr", bufs=2)
                    nc.tensor.transpose(pt[:], qr[:, k * P:(k + 1) * P], idt[:])
                    nc.scalar.copy(dst[k][:, ti * P:(ti + 1) * P], pt[:])

            def s_block(psrc, m, fcol):
                """Evacuate one accumulated S block [P, F] (rows m*P, cols
                fcol*F of s_mat); returns the fp16 staging tile."""
                st = work.tile([P, F], f16, name="sst", tag="sst", bufs=3)
                nc.vector.tensor_copy(st[:], psrc[:])
                nc.sync.dma_start(
                    s_mat[m * P:(m + 1) * P, fcol * F:(fcol + 1) * F], st[:]
                )
                return st

            def quad_group(lhs_panel, rhs_panel, m, fc):
                """One S block: rows m*P of lhs half, cols fc*F of rhs half."""
                acc = ps.tile([P, F], f32, name="acc", tag="acc", bufs=4)
                for k in range(NCH):
                    nc.tensor.matmul(
                        acc[:],
                        lhs_panel[k][:, m * P:(m + 1) * P],
                        rhs_panel[k][:, fc * F:(fc + 1) * F],
                        start=(k == 0),
                        stop=(k == NCH - 1),
                    )
                return acc

            MH = HALF // P   # 8 m-chunks per half
            FH = HALF // F   # 2 f-cols per half

            # ---- build panel A ----
            for ti in range(MH):
                build_tile(pa, 0, ti)

            # ---- S[A,A] interleaved with panel-B build ----
            for i in range(MH):
                build_tile(pb, 1, i)
                for fc in range(FH):
                    s_block(quad_group(pa, pa, i, fc), i, fc)

            # ---- S[B,B], S[A,B] (+ symmetric fill of S[B,A]) ----
            for m in range(MH):
                for fc in range(FH):
                    s_block(quad_group(pb, pb, m, fc), MH + m, FH + fc)
                for fc in range(FH):
                    st = s_block(quad_group(pa, pb, m, fc), m, FH + fc)
                    for sub in range(F // P):
                        pt = ps.tile([P, P], f16, name="tr", tag="tr", bufs=2)
                        nc.tensor.transpose(
                            pt[:], st[:, sub * P:(sub + 1) * P], idt[:]
                        )
                        ft = work.tile([P, P], f16, name="ft", tag="ft", bufs=3)
                        nc.scalar.copy(ft[:], pt[:])
                        r0 = HALF + fc * F + sub * P
                        nc.sync.dma_start(
                            s_mat[r0:r0 + P, m * P:(m + 1) * P], ft[:]
                        )

            # ---- MM2: O = S @ V (S row-panels as lhsT via symmetry) ----
            srow = []
            for k in range(NTILES):
                u = panel.tile([P, HALF], f16, name=f"pk_a{2 * k}", tag=f"pk_a{2 * k}")
                w = panel.tile(
                    [P, HALF], f16, name=f"pk_a{2 * k + 1}", tag=f"pk_a{2 * k + 1}"
                )
                nc.sync.dma_start(u[:], s_mat[k * P:(k + 1) * P, 0:HALF])
                nc.sync.dma_start(w[:], s_mat[k * P:(k + 1) * P, HALF:T])
                srow.append((u, w))

            for j in range(N // F):
                vts = []
                for k in range(NTILES):
                    vt = work.tile([P, F], f16, name=f"vt_{k}", tag=f"vt_{k}", bufs=2)
                    nc.sync.dma_start(
                        vt[:], v.ap()[k * P:(k + 1) * P, j * F:(j + 1) * F]
                    )
                    vts.append(vt)
                for m in range(NTILES):
                    acc = ps.tile([P, F], f32, name="acc", tag="acc", bufs=4)
                    for k in range(NTILES):
                        u, w = srow[k]
                        lhsT = (
                            u[:, m * P:(m + 1) * P]
                            if m < 8
                            else w[:, (m - 8) * P:(m - 7) * P]
                        )
                        nc.tensor.matmul(
                            acc[:], lhsT, vts[k][:],
                            start=(k == 0), stop=(k == NTILES - 1),
                        )
                    ot = work.tile([P, F], f32, name="ot", tag="ot", bufs=3)
                    nc.scalar.copy(ot[:], acc[:])
                    nc.sync.dma_start(
                        o.ap()[m * P:(m + 1) * P, j * F:(j + 1) * F], ot[:]
                    )

    nc.compile()
    return nc


def _tables():
    idx = np.arange(N, dtype=np.float32)
    qq = np.floor(idx / 2.0) * 2.0
    freqs = (1.0 / THETA ** (qq / N) / (2.0 * math.pi)).astype(np.float32)
    fe = freqs[::2]  # [N/2], pairs share a frequency
    ph = (np.arange(T, dtype=np.float32)[:, None] * fe[None, :]).astype(np.float32)
    ang = (np.mod(ph, 1.0) * np.float32(2.0 * math.pi)).astype(np.float32)
    cu_ = (np.cos(ang.astype(np.float64)) / 8.0).astype(np.float16)
    su_ = (np.sin(ang.astype(np.float64)) / 8.0).astype(np.float16)
    return cu_, su_


_NC_CACHE = {}


def kernel(Q, K, V, _trace=False, _tmpdir=None):
    del K  # unused by the reference computation
    if "nc" not in _NC_CACHE:
        _NC_CACHE["nc"] = _build_nc()
    nc = _NC_CACHE["nc"]

    cu_, su_ = _tables()
    ident = np.eye(P, dtype=np.float16)
    Qf = np.asarray(Q, dtype=np.float32)
    # de-interleave feature dim: [evens | odds], fp16
    Qd = np.concatenate([Qf[..., 0::2], Qf[..., 1::2]], axis=-1).astype(np.float16)
    V16 = np.asarray(V, dtype=np.float16)

    in_maps = []
    for c in range(8):
        b, h = divmod(c, NH)
        in_maps.append({
            "q": np.ascontiguousarray(Qd[b, h]),
            "v": np.ascontiguousarray(V16[b, h]),
            "cu": cu_,
            "su": su_,
            "ident": ident,
        })

    kw = {}
    if _trace:
        kw = dict(trace=True, tmpdir=_tmpdir)
    res = run_bass_kernel_spmd(nc, in_maps, list(range(8)), **kw)

    out = np.empty((B, NH, T, N), dtype=np.float32)
    for c in range(8):
        b, h = divmod(c, NH)
        out[b, h] = res.results[c]["o"]
    if _trace:
        kernel.last_exec_time_ns = res.exec_time_ns
    return out


# revision 7
# speedup vs baseline: 1.2673x; 1.0561x over previous
"""Trainium2 Bass kernel for nn_BDHAttention (RoPE(Q) self-score attention, no softmax).

Per (batch, head) slice s: QR = rope(Q_s) [T,N]; S = QR @ QR.T / sqrt(N) [T,T];
O_s = S @ V_s [T,N].  K input is unused by the reference.  B*nh = 8 slices map
1:1 onto the 8 NeuronCores (data/head parallel, no communication).

Device-side structure per core (T=2048, N=4096, P=128):
  - Q arrives fp16 with its feature dim de-interleaved on the host
    ([evens | odds]) so RoPE is all contiguous 16-bit tensor_tensor ops
    (DVE 2x mode).  The n-permutation is harmless: it is the contraction
    dim of S = QR @ QR.T and both operands share it.
  - cos/sin tables are host-precomputed fp16, scaled by 1/8 each so S picks
    up the 1/64 = 1/sqrt(N) scale for free.
  - PE-transpose QR' 128x128 tiles into two resident fp16 panels
    (QR'^T, t-halves A and B).  Panel-B build is interleaved with the
    S[A,A] matmuls to keep the PE dense (HAM stays warm).
  - MM1 (fp16, fp32 PSUM accum): S[A,A], S[B,B], S[A,B] all from resident
    panels; S[B,A] filled by PE-transposing S[A,B] blocks (S symmetric).
    S stored fp16 in a DRAM scratch.
  - MM2: O = S @ V.  S row-panels re-read from DRAM serve directly as lhsT
    tiles (partition = contraction dim) thanks to S's symmetry; V streamed
    fp16; O accumulated fp32 in PSUM and written out fp32.
"""

import math
import sys

sys.path.insert(0, "/opt/trn_rl_repo")

import numpy as np

import concourse.bacc as bacc
import concourse.mybir as mybir
import concourse.tile as tile
from concourse.bass_utils import run_bass_kernel_spmd

B, NH, T, N = 2, 4, 2048, 4096
THETA = 2 ** 16
P = 128
HALF = T // 2            # 1024
NTILES = T // P          # 16 t-tiles
NCH = N // P             # 32 n-chunks
F = 512                  # matmul moving free dim (one fp32 PSUM bank)
H = N // 2               # 2048

f16 = mybir.dt.float16
f32 = mybir.dt.float32


def _build_nc():
    nc = bacc.Bacc("TRN2", target_bir_lowering=False, debug=False, num_devices=8)

    q = nc.dram_tensor("q", [T, N], f16, kind="ExternalInput")
    v = nc.dram_tensor("v", [T, N], f16, kind="ExternalInput")
    cu = nc.dram_tensor("cu", [T, H], f16, kind="ExternalInput")
    su = nc.dram_tensor("su", [T, H], f16, kind="ExternalInput")
    ident = nc.dram_tensor("ident", [P, P], f16, kind="ExternalInput")
    o = nc.dram_tensor("o", [T, N], f32, kind="ExternalOutput")

    with tile.TileContext(nc) as tc:
        with (
            tc.tile_pool(name="dram", bufs=1, space="DRAM") as dram,
            tc.tile_pool(name="const", bufs=1) as const,
            tc.tile_pool(name="panel", bufs=1) as panel,
            tc.tile_pool(name="ps", bufs=1, space="PSUM") as ps,
            tc.tile_pool(name="work", bufs=1) as work,
        ):
            s_mat = dram.tile([T, T], f16, name="s_mat")

            idt = const.tile([P, P], f16, name="idt")
            nc.sync.dma_start(idt[:], ident.ap())

            pa = [
                panel.tile([P, HALF], f16, name=f"pk_a{k}", tag=f"pk_a{k}")
                for k in range(NCH)
            ]
            pb = [
                panel.tile([P, HALF], f16, name=f"pk_b{k}", tag=f"pk_b{k}")
                for k in range(NCH)
            ]

            def build_tile(dst, half, ti):
                """RoPE t-tile (half*8 + ti) and transpose its 32 n-chunks into
                panel columns ti*P:(ti+1)*P."""
                trow = half * (NTILES // 2) + ti
                qt = work.tile([P, N], f16, name="qt", tag="qt", bufs=1)
                cut = work.tile([P, H], f16, name="cut", tag="cut", bufs=1)
                sut = work.tile([P, H], f16, name="sut", tag="sut", bufs=1)
                nc.sync.dma_start(qt[:], q.ap()[trow * P:(trow + 1) * P, :])
                nc.sync.dma_start(cut[:], cu.ap()[trow * P:(trow + 1) * P, :])
                nc.sync.dma_start(sut[:], su.ap()[trow * P:(trow + 1) * P, :])
                qr = work.tile([P, N], f16, name="qr", tag="qr", bufs=1)
                t1 = work.tile([P, H], f16, name="t1", tag="t1", bufs=1)
                t2 = work.tile([P, H], f16, name="t2", tag="t2", bufs=1)
                qe, qo = qt[:, 0:H], qt[:, H:N]
                nc.vector.tensor_mul(t1[:], qe, cut[:])
                nc.vector.tensor_mul(t2[:], qo, sut[:])
                nc.vector.tensor_sub(qr[:, 0:H], t1[:], t2[:])
                nc.vector.tensor_mul(t1[:], qo, cut[:])
                nc.vector.tensor_mul(t2[:], qe, sut[:])
                nc.vector.tensor_add(qr[:, H:N], t1[:], t2[:])
                for k in range(NCH):
                    pt = ps.tile([P, P], f16, name="tr", tag="tr", bufs=2)
                    nc.tensor.transpose(pt[:], qr[:, k * P:(k + 1) * P], idt[:])
                    nc.scalar.copy(dst[k][:, ti * P:(ti + 1) * P], pt[:])

            def s_block(psrc, m, fcol):
                """Evacuate one accumulated S block [P, F] (rows m*P, cols
                fcol*F of s_mat); returns the fp16 staging tile."""
                st = work.tile([P, F], f16, name="sst", tag="sst", bufs=3)
                nc.vector.tensor_copy(st[:], psrc[:])
                nc.sync.dma_start(
                    s_mat[m * P:(m + 1) * P, fcol * F:(fcol + 1) * F], st[:]
                )
                return st

            def quad_group(lhs_panel, rhs_panel, m, fc):
                """One S block: rows m*P of lhs half, cols fc*F of rhs half."""
                acc = ps.tile([P, F], f32, name="acc", tag="acc", bufs=6)
                for k in range(NCH):
                    nc.tensor.matmul(
                        acc[:],
                        lhs_panel[k][:, m * P:(m + 1) * P],
                        rhs_panel[k][:, fc * F:(fc + 1) * F],
                        start=(k == 0),
                        stop=(k == NCH - 1),
                    )
                return acc

            def mirror_fill(st, row0, col0):
                """Write the transpose of block st [P, F] to s_mat rows
                row0.., cols col0.. (symmetric fill)."""
                for sub in range(F // P):
                    pt = ps.tile([P, P], f16, name="tr", tag="tr", bufs=2)
                    nc.tensor.transpose(pt[:], st[:, sub * P:(sub + 1) * P], idt[:])
                    ft = work.tile([P, P], f16, name="ft", tag="ft", bufs=3)
                    nc.scalar.copy(ft[:], pt[:])
                    r0 = row0 + sub * P
                    nc.sync.dma_start(s_mat[r0:r0 + P, col0:col0 + P], ft[:])

            def pe_warm(nmm):
                """Junk matmuls (on the const identity, so no data deps) to
                keep the PE activity monitor at full clock while the pipeline
                is otherwise DVE/DMA-bound."""
                wacc = ps.tile([P, P], f32, name="wacc", tag="tr", bufs=2)
                for i in range(nmm):
                    nc.tensor.matmul(
                        wacc[:], idt[:], idt[:],
                        start=True, stop=True, skip_group_check=True,
                    )

            MH = HALF // P   # 8 m-chunks per half
            FH = HALF // F   # 2 f-cols per half

            # ---- build panel A (junk MMs keep the PE clock warm) ----
            for ti in range(MH):
                build_tile(pa, 0, ti)
                if ti >= 1:
                    pe_warm(16)

            # ---- S[A,A] interleaved with panel-B build ----
            # skip groups entirely below the diagonal ((m,0) for m>=4);
            # they are filled from the mirror of the (m<4, fc=1) groups.
            for i in range(MH):
                build_tile(pb, 1, i)
                for fc in range(FH):
                    if fc == 0 and i >= 4:
                        continue
                    st = s_block(quad_group(pa, pa, i, fc), i, fc)
                    if fc == 1 and i < 4:
                        mirror_fill(st, F, i * P)

            # ---- S[B,B], S[A,B] (+ symmetric fills) ----
            for m in range(MH):
                for fc in range(FH):
                    if fc == 0 and m >= 4:
                        continue
                    st = s_block(quad_group(pb, pb, m, fc), MH + m, FH + fc)
                    if fc == 1 and m < 4:
                        mirror_fill(st, HALF + F, HALF + m * P)
                for fc in range(FH):
                    st = s_block(quad_group(pa, pb, m, fc), m, FH + fc)
                    mirror_fill(st, HALF + fc * F, m * P)

            # ---- MM2: O = S @ V (S row-panels as lhsT via symmetry) ----
            srow = []
            for k in range(NTILES):
                u = panel.tile([P, HALF], f16, name=f"pk_a{2 * k}", tag=f"pk_a{2 * k}")
                w = panel.tile(
                    [P, HALF], f16, name=f"pk_a{2 * k + 1}", tag=f"pk_a{2 * k + 1}"
                )
                nc.sync.dma_start(u[:], s_mat[k * P:(k + 1) * P, 0:HALF])
                nc.sync.dma_start(w[:], s_mat[k * P:(k + 1) * P, HALF:T])
                srow.append((u, w))

            for j in range(N // F):
                vts = []
                for k in range(NTILES):
                    vt = work.tile([P, F], f16, name=f"vt_{k}", tag=f"vt_{k}", bufs=2)
                    nc.sync.dma_start(
                        vt[:], v.ap()[k * P:(k + 1) * P, j * F:(j + 1) * F]
                    )
                    vts.append(vt)
                for m in range(NTILES):
                    acc = ps.tile([P, F], f32, name="acc", tag="acc", bufs=6)
                    for k in range(NTILES):
                        u, w = srow[k]
                        lhsT = (
                            u[:, m * P:(m + 1) * P]
                            if m < 8
                            else w[:, (m - 8) * P:(m - 7) * P]
                        )
                        nc.tensor.matmul(
                            acc[:], lhsT, vts[k][:],
                            start=(k == 0), stop=(k == NTILES - 1),
                        )
                    ot = work.tile([P, F], f32, name="ot", tag="ot", bufs=3)
                    nc.scalar.copy(ot[:], acc[:])
                    nc.sync.dma_start(
                        o.ap()[m * P:(m + 1) * P, j * F:(j + 1) * F], ot[:]
                    )

    nc.compile()
    return nc


def _tables():
    idx = np.arange(N, dtype=np.float32)
    qq = np.floor(idx / 2.0) * 2.0
    freqs = (1.0 / THETA ** (qq / N) / (2.0 * math.pi)).astype(np.float32)
    fe = freqs[::2]  # [N/2], pairs share a frequency
    ph = (np.arange(T, dtype=np.float32)[:, None] * fe[None, :]).astype(np.float32)
    ang = (np.mod(ph, 1.0) * np.float32(2.0 * math.pi)).astype(np.float32)
    cu_ = (np.cos(ang.astype(np.float64)) / 8.0).astype(np.float16)
    su_ = (np.sin(ang.astype(np.float64)) / 8.0).astype(np.float16)
    return cu_, su_


_NC_CACHE = {}


def kernel(Q, K, V, _trace=False, _tmpdir=None):
    del K  # unused by the reference computation
    if "nc" not in _NC_CACHE:
        _NC_CACHE["nc"] = _build_nc()
    nc = _NC_CACHE["nc"]

    cu_, su_ = _tables()
    ident = np.eye(P, dtype=np.float16)
    Qf = np.asarray(Q, dtype=np.float32)
    # de-interleave feature dim: [evens | odds], fp16
    Qd = np.concatenate([Qf[..., 0::2], Qf[..., 1::2]], axis=-1).astype(np.float16)
    V16 = np.asarray(V, dtype=np.float16)

    in_maps = []
    for c in range(8):
        b, h = divmod(c, NH)
        in_maps.append({
            "q": np.ascontiguousarray(Qd[b, h]),
            "v": np.ascontiguousarray(V16[b, h]),
            "cu": cu_,
            "su": su_,
            "ident": ident,
        })

    kw = {}
    if _trace:
        kw = dict(trace=True, tmpdir=_tmpdir)
    res = run_bass_kernel_spmd(nc, in_maps, list(range(8)), **kw)

    out = np.empty((B, NH, T, N), dtype=np.float32)
    for c in range(8):
        b, h = divmod(c, NH)
        out[b, h] = res.results[c]["o"]
    if _trace:
        kernel.last_exec_time_ns = res.exec_time_ns
    return out


# revision 8
# speedup vs baseline: 1.3345x; 1.0531x over previous
"""Trainium2 Bass kernel for nn_BDHAttention (RoPE(Q) self-score attention, no softmax).

Per (batch, head) slice s: QR = rope(Q_s) [T,N]; S = QR @ QR.T / sqrt(N) [T,T];
O_s = S @ V_s [T,N].  K input is unused by the reference.  B*nh = 8 slices map
1:1 onto the 8 NeuronCores (data/head parallel, no communication).

Device-side structure per core (T=2048, N=4096, P=128):
  - Q arrives fp16 with its feature dim de-interleaved on the host
    ([evens | odds]) so RoPE is all contiguous 16-bit tensor_tensor ops
    (DVE 2x mode).  The n-permutation is harmless: it is the contraction
    dim of S = QR @ QR.T and both operands share it.
  - cos/sin tables are host-precomputed fp16, scaled by 1/8 each so S picks
    up the 1/64 = 1/sqrt(N) scale for free.
  - PE-transpose QR' 128x128 tiles into two resident fp16 panels
    (QR'^T, t-halves A and B).  Panel-B build is interleaved with the
    S[A,A] matmuls to keep the PE dense (HAM stays warm).
  - MM1 (fp16, fp32 PSUM accum): S[A,A], S[B,B], S[A,B] all from resident
    panels; S[B,A] filled by PE-transposing S[A,B] blocks (S symmetric).
    S stored fp16 in a DRAM scratch.
  - MM2: O = S @ V.  S row-panels re-read from DRAM serve directly as lhsT
    tiles (partition = contraction dim) thanks to S's symmetry; V streamed
    fp16; O accumulated fp32 in PSUM and written out fp32.
"""

import math
import sys

sys.path.insert(0, "/opt/trn_rl_repo")

import numpy as np

import concourse.bacc as bacc
import concourse.mybir as mybir
import concourse.tile as tile
from concourse.bass_utils import run_bass_kernel_spmd

B, NH, T, N = 2, 4, 2048, 4096
THETA = 2 ** 16
P = 128
HALF = T // 2            # 1024
NTILES = T // P          # 16 t-tiles
NCH = N // P             # 32 n-chunks
F = 512                  # matmul moving free dim (one fp32 PSUM bank)
H = N // 2               # 2048

f16 = mybir.dt.float16
f32 = mybir.dt.float32


def _build_nc():
    nc = bacc.Bacc("TRN2", target_bir_lowering=False, debug=False, num_devices=8)

    q = nc.dram_tensor("q", [T, N], f16, kind="ExternalInput")
    v = nc.dram_tensor("v", [T, N], f16, kind="ExternalInput")
    cu = nc.dram_tensor("cu", [T, H], f16, kind="ExternalInput")
    su = nc.dram_tensor("su", [T, H], f16, kind="ExternalInput")
    ident = nc.dram_tensor("ident", [P, P], f16, kind="ExternalInput")
    o = nc.dram_tensor("o", [T, N], f32, kind="ExternalOutput")

    with tile.TileContext(nc) as tc:
        with (
            tc.tile_pool(name="dram", bufs=1, space="DRAM") as dram,
            tc.tile_pool(name="const", bufs=1) as const,
            tc.tile_pool(name="panel", bufs=1) as panel,
            tc.tile_pool(name="ps", bufs=1, space="PSUM") as ps,
            tc.tile_pool(name="work", bufs=1) as work,
        ):
            s_mat = dram.tile([T, T], f16, name="s_mat")

            idt = const.tile([P, P], f16, name="idt")
            nc.sync.dma_start(idt[:], ident.ap())

            pa = [
                panel.tile([P, HALF], f16, name=f"pk_a{k}", tag=f"pk_a{k}")
                for k in range(NCH)
            ]
            pb = [
                panel.tile([P, HALF], f16, name=f"pk_b{k}", tag=f"pk_b{k}")
                for k in range(NCH)
            ]

            def build_tile(dst, half, ti):
                """RoPE t-tile (half*8 + ti) and transpose its 32 n-chunks into
                panel columns ti*P:(ti+1)*P."""
                trow = half * (NTILES // 2) + ti
                qt = work.tile([P, N], f16, name="qt", tag="qt", bufs=1)
                cut = work.tile([P, H], f16, name="cut", tag="cut", bufs=1)
                sut = work.tile([P, H], f16, name="sut", tag="sut", bufs=1)
                nc.sync.dma_start(qt[:], q.ap()[trow * P:(trow + 1) * P, :])
                nc.sync.dma_start(cut[:], cu.ap()[trow * P:(trow + 1) * P, :])
                nc.sync.dma_start(sut[:], su.ap()[trow * P:(trow + 1) * P, :])
                qr = work.tile([P, N], f16, name="qr", tag="qr", bufs=1)
                t1 = work.tile([P, H], f16, name="t1", tag="t1", bufs=1)
                t2 = work.tile([P, H], f16, name="t2", tag="t2", bufs=1)
                qe, qo = qt[:, 0:H], qt[:, H:N]
                nc.vector.tensor_mul(t1[:], qe, cut[:])
                nc.vector.tensor_mul(t2[:], qo, sut[:])
                nc.vector.tensor_sub(qr[:, 0:H], t1[:], t2[:])
                nc.vector.tensor_mul(t1[:], qo, cut[:])
                nc.vector.tensor_mul(t2[:], qe, sut[:])
                nc.vector.tensor_add(qr[:, H:N], t1[:], t2[:])
                for k in range(NCH):
                    pt = ps.tile([P, P], f16, name="tr", tag="tr", bufs=2)
                    nc.tensor.transpose(pt[:], qr[:, k * P:(k + 1) * P], idt[:])
                    nc.scalar.copy(dst[k][:, ti * P:(ti + 1) * P], pt[:])

            def s_block(psrc, m, fcol):
                """Evacuate one accumulated S block [P, F] (rows m*P, cols
                fcol*F of s_mat); returns the fp16 staging tile."""
                st = work.tile([P, F], f16, name="sst", tag="sst", bufs=3)
                nc.vector.tensor_copy(st[:], psrc[:])
                nc.sync.dma_start(
                    s_mat[m * P:(m + 1) * P, fcol * F:(fcol + 1) * F], st[:]
                )
                return st

            def quad_group(lhs_panel, rhs_panel, m, fc):
                """One S block: rows m*P of lhs half, cols fc*F of rhs half."""
                acc = ps.tile([P, F], f32, name="acc", tag="acc", bufs=6)
                for k in range(NCH):
                    nc.tensor.matmul(
                        acc[:],
                        lhs_panel[k][:, m * P:(m + 1) * P],
                        rhs_panel[k][:, fc * F:(fc + 1) * F],
                        start=(k == 0),
                        stop=(k == NCH - 1),
                    )
                return acc

            def mirror_fill(st, row0, col0):
                """Write the transpose of block st [P, F] to s_mat rows
                row0.., cols col0.. (symmetric fill)."""
                for sub in range(F // P):
                    pt = ps.tile([P, P], f16, name="tr", tag="tr", bufs=2)
                    nc.tensor.transpose(pt[:], st[:, sub * P:(sub + 1) * P], idt[:])
                    ft = work.tile([P, P], f16, name="ft", tag="ft", bufs=3)
                    nc.scalar.copy(ft[:], pt[:])
                    r0 = row0 + sub * P
                    nc.sync.dma_start(s_mat[r0:r0 + P, col0:col0 + P], ft[:])

            def pe_warm(nmm):
                """Junk matmuls (on the const identity, so no data deps) to
                keep the PE activity monitor at full clock while the pipeline
                is otherwise DVE/DMA-bound."""
                wacc = ps.tile([P, P], f32, name="wacc", tag="tr", bufs=2)
                for i in range(nmm):
                    nc.tensor.matmul(
                        wacc[:], idt[:], idt[:],
                        start=True, stop=True, skip_group_check=True,
                    )

            MH = HALF // P   # 8 m-chunks per half
            FH = HALF // F   # 2 f-cols per half

            # ---- build panel A (junk MMs keep the PE clock warm) ----
            pe_warm(8)
            for ti in range(MH):
                build_tile(pa, 0, ti)
                pe_warm(32)

            # ---- S[A,A] interleaved with panel-B build ----
            # skip groups entirely below the diagonal ((m,0) for m>=4);
            # they are filled from the mirror of the (m<4, fc=1) groups.
            for i in range(MH):
                build_tile(pb, 1, i)
                for fc in range(FH):
                    if fc == 0 and i >= 4:
                        continue
                    st = s_block(quad_group(pa, pa, i, fc), i, fc)
                    if fc == 1 and i < 4:
                        mirror_fill(st, F, i * P)

            # ---- S[B,B], S[A,B] (+ symmetric fills) ----
            pe_warm(16)
            for m in range(MH):
                for fc in range(FH):
                    if fc == 0 and m >= 4:
                        continue
                    st = s_block(quad_group(pb, pb, m, fc), MH + m, FH + fc)
                    if fc == 1 and m < 4:
                        mirror_fill(st, HALF + F, HALF + m * P)
                for fc in range(FH):
                    st = s_block(quad_group(pa, pb, m, fc), m, FH + fc)
                    mirror_fill(st, HALF + fc * F, m * P)

            # ---- MM2: O = S @ V (S row-panels as lhsT via symmetry) ----
            srow = []
            for k in range(NTILES):
                u = panel.tile([P, HALF], f16, name=f"pk_a{2 * k}", tag=f"pk_a{2 * k}")
                w = panel.tile(
                    [P, HALF], f16, name=f"pk_a{2 * k + 1}", tag=f"pk_a{2 * k + 1}"
                )
                nc.sync.dma_start(u[:], s_mat[k * P:(k + 1) * P, 0:HALF])
                nc.sync.dma_start(w[:], s_mat[k * P:(k + 1) * P, HALF:T])
                srow.append((u, w))

            pe_warm(24)
            for j in range(N // F):
                vts = []
                for k in range(NTILES):
                    vt = work.tile([P, F], f16, name=f"vt_{k}", tag=f"vt_{k}", bufs=2)
                    nc.sync.dma_start(
                        vt[:], v.ap()[k * P:(k + 1) * P, j * F:(j + 1) * F]
                    )
                    vts.append(vt)
                for m in range(NTILES):
                    acc = ps.tile([P, F], f32, name="acc", tag="acc", bufs=6)
                    for k in range(NTILES):
                        u, w = srow[k]
                        lhsT = (
                            u[:, m * P:(m + 1) * P]
                            if m < 8
                            else w[:, (m - 8) * P:(m - 7) * P]
                        )
                        nc.tensor.matmul(
                            acc[:], lhsT, vts[k][:],
                            start=(k == 0), stop=(k == NTILES - 1),
                        )
                    ot = work.tile([P, F], f32, name="ot", tag="ot", bufs=3)
                    nc.scalar.copy(ot[:], acc[:])
                    nc.sync.dma_start(
                        o.ap()[m * P:(m + 1) * P, j * F:(j + 1) * F], ot[:]
                    )

    nc.compile()
    return nc


def _tables():
    idx = np.arange(N, dtype=np.float32)
    qq = np.floor(idx / 2.0) * 2.0
    freqs = (1.0 / THETA ** (qq / N) / (2.0 * math.pi)).astype(np.float32)
    fe = freqs[::2]  # [N/2], pairs share a frequency
    ph = (np.arange(T, dtype=np.float32)[:, None] * fe[None, :]).astype(np.float32)
    ang = (np.mod(ph, 1.0) * np.float32(2.0 * math.pi)).astype(np.float32)
    cu_ = (np.cos(ang.astype(np.float64)) / 8.0).astype(np.float16)
    su_ = (np.sin(ang.astype(np.float64)) / 8.0).astype(np.float16)
    return cu_, su_


_NC_CACHE = {}


def kernel(Q, K, V, _trace=False, _tmpdir=None):
    del K  # unused by the reference computation
    if "nc" not in _NC_CACHE:
        _NC_CACHE["nc"] = _build_nc()
    nc = _NC_CACHE["nc"]

    cu_, su_ = _tables()
    ident = np.eye(P, dtype=np.float16)
    Qf = np.asarray(Q, dtype=np.float32)
    # de-interleave feature dim: [evens | odds], fp16
    Qd = np.concatenate([Qf[..., 0::2], Qf[..., 1::2]], axis=-1).astype(np.float16)
    V16 = np.asarray(V, dtype=np.float16)

    in_maps = []
    for c in range(8):
        b, h = divmod(c, NH)
        in_maps.append({
            "q": np.ascontiguousarray(Qd[b, h]),
            "v": np.ascontiguousarray(V16[b, h]),
            "cu": cu_,
            "su": su_,
            "ident": ident,
        })

    kw = {}
    if _trace:
        kw = dict(trace=True, tmpdir=_tmpdir)
    res = run_bass_kernel_spmd(nc, in_maps, list(range(8)), **kw)

    out = np.empty((B, NH, T, N), dtype=np.float32)
    for c in range(8):
        b, h = divmod(c, NH)
        out[b, h] = res.results[c]["o"]
    if _trace:
        kernel.last_exec_time_ns = res.exec_time_ns
    return out


# revision 12
# speedup vs baseline: 1.3694x; 1.0261x over previous
"""Trainium2 Bass kernel for nn_BDHAttention (RoPE(Q) self-score attention, no softmax).

Per (batch, head) slice s: QR = rope(Q_s) [T,N]; S = QR @ QR.T / sqrt(N) [T,T];
O_s = S @ V_s [T,N].  K input is unused by the reference.  B*nh = 8 slices map
1:1 onto the 8 NeuronCores (data/head parallel, no communication).

Device-side structure per core (T=2048, N=4096, P=128):
  - Q arrives fp16 with its feature dim de-interleaved on the host
    ([evens | odds]) so RoPE is all contiguous 16-bit tensor_tensor ops
    (DVE 2x mode).  The n-permutation is harmless: it is the contraction
    dim of S = QR @ QR.T and both operands share it.
  - cos/sin tables are host-precomputed fp16, scaled by 1/8 each so S picks
    up the 1/64 = 1/sqrt(N) scale for free.
  - PE-transpose QR' 128x128 tiles into two resident fp16 panels
    (QR'^T, t-halves A and B).  Panel-B build is interleaved with the
    S[A,A] matmuls to keep the PE dense (HAM stays warm).
  - MM1 (fp16, fp32 PSUM accum): S[A,A], S[B,B], S[A,B] all from resident
    panels; S[B,A] filled by PE-transposing S[A,B] blocks (S symmetric).
    S stored fp16 in a DRAM scratch.
  - MM2: O = S @ V.  S row-panels re-read from DRAM serve directly as lhsT
    tiles (partition = contraction dim) thanks to S's symmetry; V streamed
    fp16; O accumulated fp32 in PSUM and written out fp32.
"""

import math
import sys

sys.path.insert(0, "/opt/trn_rl_repo")

import numpy as np

import concourse.bacc as bacc
import concourse.mybir as mybir
import concourse.tile as tile
from concourse.bass_utils import run_bass_kernel_spmd

B, NH, T, N = 2, 4, 2048, 4096
THETA = 2 ** 16
P = 128
HALF = T // 2            # 1024
NTILES = T // P          # 16 t-tiles
NCH = N // P             # 32 n-chunks
F = 512                  # matmul moving free dim (one fp32 PSUM bank)
H = N // 2               # 2048

f16 = mybir.dt.float16
f32 = mybir.dt.float32


def _build_nc():
    nc = bacc.Bacc("TRN2", target_bir_lowering=False, debug=False, num_devices=8)

    q = nc.dram_tensor("q", [T, N], f16, kind="ExternalInput")
    v = nc.dram_tensor("v", [T, N], f16, kind="ExternalInput")
    cu = nc.dram_tensor("cu", [T, H], f16, kind="ExternalInput")
    su = nc.dram_tensor("su", [T, H], f16, kind="ExternalInput")
    ident = nc.dram_tensor("ident", [P, P], f16, kind="ExternalInput")
    o = nc.dram_tensor("o", [T, N], f32, kind="ExternalOutput")

    with tile.TileContext(nc) as tc:
        with (
            tc.tile_pool(name="dram", bufs=1, space="DRAM") as dram,
            tc.tile_pool(name="const", bufs=1) as const,
            tc.tile_pool(name="panel", bufs=1) as panel,
            tc.tile_pool(name="ps", bufs=1, space="PSUM") as ps,
            tc.tile_pool(name="work", bufs=1) as work,
        ):
            s_mat = dram.tile([T, T], f16, name="s_mat")

            idt = const.tile([P, P], f16, name="idt")
            nc.sync.dma_start(idt[:], ident.ap())

            pa = [
                panel.tile([P, HALF], f16, name=f"pk_a{k}", tag=f"pk_a{k}")
                for k in range(NCH)
            ]
            pb = [
                panel.tile([P, HALF], f16, name=f"pk_b{k}", tag=f"pk_b{k}")
                for k in range(NCH)
            ]

            def build_tile(dst, half, ti):
                """RoPE t-tile (half*8 + ti) and transpose its 32 n-chunks into
                panel columns ti*P:(ti+1)*P."""
                trow = half * (NTILES // 2) + ti
                qt = work.tile([P, N], f16, name="qt", tag="qt", bufs=1)
                cut = work.tile([P, H], f16, name="cut", tag="cut", bufs=1)
                sut = work.tile([P, H], f16, name="sut", tag="sut", bufs=1)
                nc.sync.dma_start(qt[:], q.ap()[trow * P:(trow + 1) * P, :])
                nc.sync.dma_start(cut[:], cu.ap()[trow * P:(trow + 1) * P, :])
                nc.sync.dma_start(sut[:], su.ap()[trow * P:(trow + 1) * P, :])
                qr = work.tile([P, N], f16, name="qr", tag="qr", bufs=1)
                t1 = work.tile([P, H], f16, name="t1", tag="t1", bufs=1)
                t2 = work.tile([P, H], f16, name="t2", tag="t2", bufs=1)
                qe, qo = qt[:, 0:H], qt[:, H:N]
                nc.vector.tensor_mul(t1[:], qe, cut[:])
                nc.vector.tensor_mul(t2[:], qo, sut[:])
                nc.vector.tensor_sub(qr[:, 0:H], t1[:], t2[:])
                nc.vector.tensor_mul(t1[:], qo, cut[:])
                nc.vector.tensor_mul(t2[:], qe, sut[:])
                nc.vector.tensor_add(qr[:, H:N], t1[:], t2[:])
                for k in range(NCH):
                    pt = ps.tile([P, P], f16, name="tr", tag="tr", bufs=2)
                    nc.tensor.transpose(pt[:], qr[:, k * P:(k + 1) * P], idt[:])
                    nc.scalar.copy(dst[k][:, ti * P:(ti + 1) * P], pt[:])

            def s_block(psrc, row, col, width):
                """Evacuate one accumulated S block [P, width] to s_mat rows
                row.., cols col..; returns the fp16 staging tile."""
                st = work.tile([P, width], f16, name="sst", tag="sst", bufs=3)
                nc.vector.tensor_copy(st[:], psrc[:])
                nc.sync.dma_start(s_mat[row:row + P, col:col + width], st[:])
                return st

            def quad_group(lhs_panel, rhs_panel, m, c0, width):
                """One S block: rows m*P of lhs half, cols [c0, c0+width) of
                rhs half (element offsets)."""
                acc = ps.tile([P, width], f32, name="acc", tag="acc", bufs=6)
                for k in range(NCH):
                    nc.tensor.matmul(
                        acc[:],
                        lhs_panel[k][:, m * P:(m + 1) * P],
                        rhs_panel[k][:, c0:c0 + width],
                        start=(k == 0),
                        stop=(k == NCH - 1),
                    )
                return acc

            def mirror_one(st, sub, r0, c0):
                """Write the transpose of st's sub-block [P, P] (cols sub*P..)
                to s_mat rows r0.., cols c0.. (symmetric fill)."""
                pt = ps.tile([P, P], f16, name="tr", tag="tr", bufs=2)
                nc.tensor.transpose(pt[:], st[:, sub * P:(sub + 1) * P], idt[:])
                ft = work.tile([P, P], f16, name="ft", tag="ft", bufs=3)
                nc.scalar.copy(ft[:], pt[:])
                nc.sync.dma_start(s_mat[r0:r0 + P, c0:c0 + P], ft[:])

            def diag_quadrant_row(pan, q0, m):
                """Row-chunk m of a diagonal quadrant (origin q0 in s_mat):
                compute only blocks on/above the diagonal; mirror-fill the
                strictly-above blocks into the skipped mirror positions."""
                for fc in range(FH):
                    j0 = max(0, m - 4 * fc)
                    if j0 >= F // P:
                        continue
                    width = (F // P - j0) * P
                    c0 = fc * F + j0 * P
                    acc = quad_group(pan, pan, m, c0, width)
                    st = s_block(acc, q0 + m * P, q0 + c0, width)
                    for sub in range(width // P):
                        c = 4 * fc + j0 + sub
                        if c > m:
                            mirror_one(st, sub, q0 + c * P, q0 + m * P)

            def pe_warm(nmm):
                """Junk matmuls (on the const identity, so no data deps) to
                keep the PE activity monitor at full clock while the pipeline
                is otherwise DVE/DMA-bound."""
                wacc = ps.tile([P, P], f32, name="wacc", tag="tr", bufs=2)
                for i in range(nmm):
                    nc.tensor.matmul(
                        wacc[:], idt[:], idt[:],
                        start=True, stop=True, skip_group_check=True,
                    )

            MH = HALF // P   # 8 m-chunks per half
            FH = HALF // F   # 2 f-cols per half

            # ---- build panel A (junk MMs keep the PE clock warm) ----
            pe_warm(8)
            for ti in range(MH):
                build_tile(pa, 0, ti)
                pe_warm(32)

            # ---- S[A,A] (diag-block skipping) interleaved with panel-B build ----
            for i in range(MH):
                build_tile(pb, 1, i)
                diag_quadrant_row(pa, 0, i)

            # ---- S[B,B] (diag-block skipping), S[A,B] (+ mirror to S[B,A]) ----
            pe_warm(16)
            for m in range(MH):
                diag_quadrant_row(pb, HALF, m)
                for fc in range(FH):
                    acc = quad_group(pa, pb, m, fc * F, F)
                    st = s_block(acc, m * P, HALF + fc * F, F)
                    for sub in range(F // P):
                        mirror_one(st, sub, HALF + fc * F + sub * P, m * P)

            # ---- MM2: O = S @ V (S row-panels as lhsT via symmetry) ----
            srow = []
            for k in range(NTILES):
                u = panel.tile([P, HALF], f16, name=f"pk_a{2 * k}", tag=f"pk_a{2 * k}")
                w = panel.tile(
                    [P, HALF], f16, name=f"pk_a{2 * k + 1}", tag=f"pk_a{2 * k + 1}"
                )
                nc.sync.dma_start(u[:], s_mat[k * P:(k + 1) * P, 0:HALF])
                nc.sync.dma_start(w[:], s_mat[k * P:(k + 1) * P, HALF:T])
                srow.append((u, w))

            pe_warm(24)
            for j in range(N // F):
                vts = []
                for k in range(NTILES):
                    vt = work.tile([P, F], f16, name=f"vt_{k}", tag=f"vt_{k}", bufs=2)
                    nc.sync.dma_start(
                        vt[:], v.ap()[k * P:(k + 1) * P, j * F:(j + 1) * F]
                    )
                    vts.append(vt)
                for m in range(NTILES):
                    acc = ps.tile([P, F], f32, name="acc", tag="acc", bufs=6)
                    for k in range(NTILES):
                        u, w = srow[k]
                        lhsT = (
                            u[:, m * P:(m + 1) * P]
                            if m < 8
                            else w[:, (m - 8) * P:(m - 7) * P]
                        )
                        nc.tensor.matmul(
                            acc[:], lhsT, vts[k][:],
                            start=(k == 0), stop=(k == NTILES - 1),
                        )
                    ot = work.tile([P, F], f32, name="ot", tag="ot", bufs=3)
                    nc.scalar.copy(ot[:], acc[:])
                    nc.sync.dma_start(
                        o.ap()[m * P:(m + 1) * P, j * F:(j + 1) * F], ot[:]
                    )

    nc.compile()
    return nc


def _tables():
    idx = np.arange(N, dtype=np.float32)
    qq = np.floor(idx / 2.0) * 2.0
    freqs = (1.0 / THETA ** (qq / N) / (2.0 * math.pi)).astype(np.float32)
    fe = freqs[::2]  # [N/2], pairs share a frequency
    ph = (np.arange(T, dtype=np.float32)[:, None] * fe[None, :]).astype(np.float32)
    ang = (np.mod(ph, 1.0) * np.float32(2.0 * math.pi)).astype(np.float32)
    cu_ = (np.cos(ang.astype(np.float64)) / 8.0).astype(np.float16)
    su_ = (np.sin(ang.astype(np.float64)) / 8.0).astype(np.float16)
    return cu_, su_


_NC_CACHE = {}


def kernel(Q, K, V, _trace=False, _tmpdir=None):
    del K  # unused by the reference computation
    if "nc" not in _NC_CACHE:
        _NC_CACHE["nc"] = _build_nc()
    nc = _NC_CACHE["nc"]

    cu_, su_ = _tables()
    ident = np.eye(P, dtype=np.float16)
    Qf = np.asarray(Q, dtype=np.float32)
    # de-interleave feature dim: [evens | odds], fp16
    Qd = np.concatenate([Qf[..., 0::2], Qf[..., 1::2]], axis=-1).astype(np.float16)
    V16 = np.asarray(V, dtype=np.float16)

    in_maps = []
    for c in range(8):
        b, h = divmod(c, NH)
        in_maps.append({
            "q": np.ascontiguousarray(Qd[b, h]),
            "v": np.ascontiguousarray(V16[b, h]),
            "cu": cu_,
            "su": su_,
            "ident": ident,
        })

    kw = {}
    if _trace:
        kw = dict(trace=True, tmpdir=_tmpdir)
    res = run_bass_kernel_spmd(nc, in_maps, list(range(8)), **kw)

    out = np.empty((B, NH, T, N), dtype=np.float32)
    for c in range(8):
        b, h = divmod(c, NH)
        out[b, h] = res.results[c]["o"]
    if _trace:
        kernel.last_exec_time_ns = res.exec_time_ns
    return out
